# revision 27
# baseline (speedup 1.0000x reference)
"""2-layer GCN (EvolveGCN eval forward) on 8 Trainium2 NeuronCores.

Strategy (graph/data parallel, per sharding hint):
  - Nodes are assigned to 8 cores x 49 blocks x 128 slots by a host-side
    balanced bin-packing (equalizes per-(core,block) edge load for both
    src-index windows, so the SPMD-uniform per-block run counts carry
    minimal padding). Each core owns the edges whose *dst* lands in its
    slots and produces those output rows.
  - GCN layer is factored as  out = dinv * (segsum(hs[src]) + hs) + b  with
    hs = dinv * (X @ W), so no per-edge FP math is needed beyond the
    segment-sum itself.
  - Per core/layer: dense X@W on TensorE; hs rows are AllGathered across
    cores (bf16 rows in shared DRAM); per-dst-block aggregation runs as
    [128-edge] x [one-hot S] matmuls accumulating in PSUM; LayerNorm+ReLU
    epilogue on VectorE/ScalarE.
  - The per-edge gather of hs[src] uses the GPSIMD dma_gather custom DMA
    (int16 indices; the padded node space 50176 is split in two 25088-row
    windows so indices fit in int16). Gather chunks are striped across 4
    SWDGE queues (one DMA ring each) with deep prefetch: a single ring is
    latency/bandwidth-bound at ~10 ns per 256 B row, so 4 queues give ~4x.
    Edges are sorted by src slot within each run for HBM page locality,
    and padded lanes point at spread-out rows to avoid bank hammering.

Host-side work is integer graph preprocessing only (edge bucketing/sorting,
balanced slot assignment, index relabeling, layout transposes); all
floating-point model math runs on device.
"""
import sys

sys.path.insert(0, "/opt/trn_rl_repo")

import numpy as np
import ml_dtypes

import concourse.bacc as bacc
import concourse.bass as bass
import concourse.mybir as mybir
import concourse.tile as tile
from concourse.tile import add_dep_helper
from concourse.bass_utils import run_bass_kernel_spmd
from concourse.library_config import mlp as _mlp_lib
from concourse._compat import cdiv

P = 128
NCORES = 8
N_NODES = 50000
PART = N_NODES // NCORES          # 6250 real nodes per core
NB = cdiv(PART, P)                # 49 dst blocks per core
PPAD = NB * P                     # 6272 padded nodes per core
NPAD = NCORES * PPAD              # 50176 padded global nodes
SPLIT = NPAD // 2                 # 25088 (< 32768 so int16 indices work)
F = 128                           # feature dim
LN_EPS = 1e-5
RUNS_PER_CHUNK = 32               # 32 runs * 128 edges = 4096 gathers/call
GPOOL_BUFS = 1                    # per-tag bufs set at tile() time
SPOOL_BUFS = 16
N_GSEMS = 8
N_QUEUES = 4                      # SWDGE queues; chunks striped across them


# ---------------------------------------------------------------- host prep
def _wrap16(arr):
    """[L] int -> [128, L/16] int16, index j of the stream at [j%16, j//16],
    replicated to all 8 GPSIMD core partition groups."""
    L = arr.shape[0]
    assert L % 16 == 0
    a2 = arr.reshape(L // 16, 16).T          # [16, S]
    return np.ascontiguousarray(np.tile(a2, (8, 1)).astype(np.int16))


def _balance_slots(a_n, b_n):
    """Assign each node a padded slot, balancing per-(core,block) A/B edge
    loads. Nodes < N_NODES//2 go to slots < SPLIT (cores 0-3) so the A/B
    src-window classification stays consistent. Returns slot_of [N_NODES]."""
    half = N_NODES // 2
    NBINS = NCORES * NB
    A = np.zeros(NBINS, dtype=np.int64)
    B = np.zeros(NBINS, dtype=np.int64)
    CNT = np.zeros(NBINS, dtype=np.int64)
    slot_of = np.empty(N_NODES, dtype=np.int64)
    half_bins = NBINS // 2
    deg = a_n + b_n
    for lo, hi, bins in ((0, half, slice(0, half_bins)),
                         (half, N_NODES, slice(half_bins, NBINS))):
        nodes = np.arange(lo, hi)
        order = nodes[np.argsort(-deg[lo:hi], kind="stable")]
        Av, Bv, Cv = A[bins], B[bins], CNT[bins]
        base = (bins.start if bins.start else 0)
        cap = 6.0 * P  # 768: keep both streams at <=6 runs per block
        for n in order:
            An, Bn = Av + a_n[n], Bv + b_n[n]
            score = (np.maximum(An - cap, 0.0) ** 2
                     + np.maximum(Bn - cap, 0.0) ** 2
                     + 1e-6 * (An * An + Bn * Bn))
            score[Cv >= P] = np.inf
            i = int(np.argmin(score))
            slot_of[n] = (base + i) * P + Cv[i]
            Av[i] += a_n[n]
            Bv[i] += b_n[n]
            Cv[i] += 1
    return slot_of


def _prep(x, edge_index):
    """Integer-only graph preprocessing + layout prep. Returns
    (per-core input dict list, MA, MB, RA, RB, slot_of) where MA/MB are
    per-block run counts (uniform across cores -> baked into the program)
    and slot_of maps node id -> padded slot (for x/out permutation)."""
    src = np.asarray(edge_index[0], dtype=np.int64)
    dst = np.asarray(edge_index[1], dtype=np.int64)

    half = N_NODES // 2
    a_cnt = np.bincount(dst[src < half], minlength=N_NODES)
    b_cnt = np.bincount(dst[src >= half], minlength=N_NODES)
    slot_of = _balance_slots(a_cnt, b_cnt)

    counts = (a_cnt + b_cnt).astype(np.float32)

    sd = slot_of[dst]
    core_of = sd // PPAD
    blk = (sd % PPAD) >> 7                   # 0..48
    dr = sd & (P - 1)                        # position within block
    ps = slot_of[src]                        # padded global src slot
    is_b = (ps >= SPLIT).astype(np.int64)

    order = np.lexsort((ps, is_b, blk, core_of))
    ps_s, dr_s, blk_s, core_s, isb_s = (
        ps[order], dr[order], blk[order], core_of[order], is_b[order])

    # group sizes per (core, blk, range)
    key = (core_s * NB + blk_s) * 2 + isb_s
    sizes = np.bincount(key, minlength=NCORES * NB * 2).reshape(NCORES, NB, 2)
    MA = np.max(np.ceil(sizes[:, :, 0] / P).astype(np.int64), axis=0)  # [NB]
    MB = np.max(np.ceil(sizes[:, :, 1] / P).astype(np.int64), axis=0)  # [NB]
    posA = np.concatenate([[0], np.cumsum(MA)])   # run offset of block b in A stream
    posB = np.concatenate([[0], np.cumsum(MB)])
    RA, RB = int(posA[-1]), int(posB[-1])

    # per-core edge segments (cumulative offsets into the sorted arrays)
    seg_end = np.cumsum(sizes.reshape(-1))
    seg_start = seg_end - sizes.reshape(-1)
    seg_start = seg_start.reshape(NCORES, NB, 2)
    seg_len = sizes

    # spread padding indices over distinct rows (avoid HBM bank hammering)
    padA = (np.arange(RA * P, dtype=np.int64) * 37) % SPLIT
    padB = (np.arange(RB * P, dtype=np.int64) * 37) % (NPAD - SPLIT)

    in_maps = []
    for c in range(NCORES):
        idxA = padA.copy()
        drA = np.full(RA * P, -1.0, dtype=np.float32)
        idxB = padB.copy()
        drB = np.full(RB * P, -1.0, dtype=np.float32)
        for b in range(NB):
            s0, n0 = seg_start[c, b, 0], seg_len[c, b, 0]
            o = posA[b] * P
            idxA[o:o + n0] = ps_s[s0:s0 + n0]
            drA[o:o + n0] = dr_s[s0:s0 + n0].astype(np.float32)
            s1, n1 = seg_start[c, b, 1], seg_len[c, b, 1]
            o = posB[b] * P
            idxB[o:o + n1] = ps_s[s1:s1 + n1] - SPLIT
            drB[o:o + n1] = dr_s[s1:s1 + n1].astype(np.float32)

        cnt = np.zeros(NPAD, dtype=np.float32)
        cnt[slot_of] = counts
        cnt = cnt[c * PPAD:(c + 1) * PPAD]
        xs = np.zeros((NPAD, F), dtype=np.float32)
        xs[slot_of] = np.asarray(x, dtype=np.float32)
        xT = np.ascontiguousarray(
            xs[c * PPAD:(c + 1) * PPAD].T).astype(ml_dtypes.bfloat16)

        m = {
            "xT": np.ascontiguousarray(xT),
            "cnts": np.ascontiguousarray(cnt.reshape(NB, P).T),       # [128, NB]
            "idxA": _wrap16(idxA) if RA else np.zeros((128, 1), np.int16),
            "idxB": _wrap16(idxB) if RB else np.zeros((128, 1), np.int16),
            "drA": (np.ascontiguousarray(drA.reshape(RA, P).T).astype(ml_dtypes.bfloat16)
                    if RA else np.zeros((128, 1), ml_dtypes.bfloat16)),
            "drB": (np.ascontiguousarray(drB.reshape(RB, P).T).astype(ml_dtypes.bfloat16)
                    if RB else np.zeros((128, 1), ml_dtypes.bfloat16)),
        }
        in_maps.append(m)
    return in_maps, MA.tolist(), MB.tolist(), RA, RB, slot_of


# ---------------------------------------------------------------- device code
def _inst(x):
    # unwrap BassInstruction -> mybir.Instruction; raw Instructions pass through
    # (note: Instruction.ins is its *operand list*, so type-check, don't duck-type)
    import bass_rust as _br
    if isinstance(x, _br.Instruction):
        return x
    return x.ins


class _Gather:
    """Streams one range's gather chunks through a tile pool with software
    prefetch: the dma_gather issue and the data-arrival wait live in separate
    critical sections so descriptor generation and DMA of later chunks overlap
    the wait for the current one. Consumers must register via dep()."""

    PREFETCH = 4

    def __init__(self, nc, tc, pool, sems, sem_counts, idx_t, src_ap, n_runs, name,
                 qrr=None):
        self.nc, self.tc, self.pool = nc, tc, pool
        self.sems, self.sem_counts = sems, sem_counts
        self.idx_t, self.src_ap, self.n_runs = idx_t, src_ap, n_runs
        self.n_chunks = cdiv(n_runs, RUNS_PER_CHUNK)
        self.name = name
        self.qrr = qrr if qrr is not None else [0]   # shared round-robin counter
        self.issued = -1
        self.sem_i = 0
        self.tiles = {}
        self.gi = {}        # chunk -> gather inst
        self.wi = {}        # chunk -> wait inst
        self.semtgt = {}

    def _emit_issue(self, ci):
        nruns = min(RUNS_PER_CHUNK, self.n_runs - ci * RUNS_PER_CHUNK)
        n = nruns * P
        off = ci * RUNS_PER_CHUNK * P
        g = self.pool.tile([P, RUNS_PER_CHUNK, F], mybir.dt.bfloat16,
                           tag=f"gchunk_{self.name[:1]}", bufs=self.PREFETCH + 2,
                           name=f"g_{self.name}_{ci}")
        k = self.sem_i % len(self.sems)
        self.sem_i += 1
        sem = self.sems[k]
        self.sem_counts[k] += 16
        self.semtgt[ci] = (sem, self.sem_counts[k])
        qn = self.qrr[0] % N_QUEUES
        self.qrr[0] += 1
        gi = self.nc.gpsimd.dma_gather(
            g[:, :nruns, :], self.src_ap,
            self.idx_t[:, off // 16:(off + n) // 16],
            n, n, F, single_packet=False, queue_num=qn,
        ).then_inc(sem, 16)
        self.tiles[ci] = g
        self.gi[ci] = gi
        self.issued = ci

    def rhs(self, r):
        """Returns (rhs AP for run r, chunk index). Caller must dep() every
        consumer instruction on that chunk. One critical section per chunk
        advance: {issue chunk ci+PREFETCH; wait chunk ci} -- sections chain
        in emission order on gpsimd, so issues always precede their waits."""
        ci = r // RUNS_PER_CHUNK
        if ci not in self.wi:
            with self.tc.tile_critical(no_gpsimd_drain=True):
                while self.issued < min(ci + self.PREFETCH, self.n_chunks - 1):
                    self._emit_issue(self.issued + 1)
                sem, tgt = self.semtgt[ci]
                self.nc.gpsimd.wait_ge(sem, tgt)
            self.wi[ci] = self.tc.prev_crit_insts[mybir.EngineType.Pool]
            self.tiles.pop(ci - 1, None)
        return self.tiles[ci][:, r - ci * RUNS_PER_CHUNK, :], ci

    def dep(self, inst, ci):
        add_dep_helper(_inst(inst), _inst(self.wi[ci]), sync=True,
                       reason=f"gather consume {self.name}:{ci}")


def _build(MA, MB, RA, RB, trivial_affine, for_sim=False, reps=1, no_gather=False,
           ag_mode="coll", probe_reloads=0):
    """Build + compile the 8-core SPMD program. trivial_affine: b==0, g==1,
    be==0 for both layers (checked on host) -> skip those tensor ops.
    for_sim: single-core cost-model variant (AllGather replaced by a local
    DRAM copy so TimelineSim can run it)."""
    posA = np.concatenate([[0], np.cumsum(MA)]).astype(int)
    posB = np.concatenate([[0], np.cumsum(MB)]).astype(int)

    nc = bacc.Bacc("TRN2", target_bir_lowering=False, debug=False,
                   num_devices=1 if for_sim else NCORES,
                   num_swdge_queues=N_QUEUES)
    qrr = [0]
    f32 = mybir.dt.float32
    bf16 = mybir.dt.bfloat16

    # ---- I/O
    xT_d = nc.dram_tensor("xT", [F, PPAD], mybir.dt.bfloat16, kind="ExternalInput")
    cnts_d = nc.dram_tensor("cnts", [P, NB], f32, kind="ExternalInput")
    idxA_d = nc.dram_tensor("idxA", [P, max(RA * 8, 1)], mybir.dt.int16, kind="ExternalInput")
    idxB_d = nc.dram_tensor("idxB", [P, max(RB * 8, 1)], mybir.dt.int16, kind="ExternalInput")
    drA_d = nc.dram_tensor("drA", [P, max(RA, 1)], mybir.dt.bfloat16, kind="ExternalInput")
    drB_d = nc.dram_tensor("drB", [P, max(RB, 1)], mybir.dt.bfloat16, kind="ExternalInput")
    w_d = [nc.dram_tensor(f"W{l+1}", [F, F], mybir.dt.bfloat16, kind="ExternalInput") for l in range(2)]
    aff_d = []
    if not trivial_affine:
        for l in range(2):
            aff_d.append({k: nc.dram_tensor(f"{k}{l+1}", [P, F], f32, kind="ExternalInput")
                          for k in ("bB", "gB", "beB")})
    ident_d = nc.dram_tensor("ident", [P, P], f32, kind="ExternalInput")
    identb_d = nc.dram_tensor("identb", [P, P], mybir.dt.bfloat16, kind="ExternalInput")
    colio_d = nc.dram_tensor("colio", [P, P], mybir.dt.bfloat16, kind="ExternalInput")
    out_d = nc.dram_tensor("out", [PPAD, F], f32, kind="ExternalOutput")

    shared_ag = (ag_mode == "shared") and not for_sim

    import contextlib
    with tile.TileContext(nc) as tc, contextlib.ExitStack() as st:
        gsems = [st.enter_context(nc.semaphore(f"gsem{i}")) for i in range(N_GSEMS)]
        sem_counts = [0] * N_GSEMS
        wsems = agsems = None
        wsem_tgt = [0, 0]
        agsem_tgt = [0, 0]
        spid = [None]
        if shared_ag:
            wsems = [st.enter_context(nc.semaphore(f"wsem{l}")) for l in range(2)]
            agsems = [st.enter_context(nc.semaphore(f"agsem{l}")) for l in range(2)]
        pers = st.enter_context(tc.tile_pool(name="pers", bufs=1))
        gpool = st.enter_context(tc.tile_pool(name="gpool", bufs=GPOOL_BUFS))
        spool = st.enter_context(tc.tile_pool(name="spool", bufs=SPOOL_BUFS))
        tpool = st.enter_context(tc.tile_pool(name="tpool", bufs=4))
        vpool = st.enter_context(tc.tile_pool(name="vpool", bufs=8))
        ph_ps = st.enter_context(tc.tile_pool(name="ph_ps", bufs=2, space="PSUM"))
        ag_ps = st.enter_context(tc.tile_pool(name="ag_ps", bufs=5, space="PSUM"))
        tr_ps = st.enter_context(tc.tile_pool(name="tr_ps", bufs=1, space="PSUM"))
        dram = st.enter_context(tc.tile_pool(name="dram", bufs=1, space="DRAM"))

        nc.gpsimd.load_library(_mlp_lib)
        if shared_ag:
            spid[0] = nc.sync.partition_id()

        # ---- persistent loads
        def load(name, dten, shape, dt=None):
            t = pers.tile(shape, dt or f32, name=name)
            nc.sync.dma_start(out=t[:], in_=dten[:])
            return t

        xT = load("xT_t", xT_d, [F, PPAD], bf16)
        cnts = load("cnts_t", cnts_d, [P, NB])
        w_t = [load(f"w{l}_t", w_d[l], [F, F], bf16) for l in range(2)]
        ident = load("ident_t", ident_d, [P, P])
        identb = load("identb_t", identb_d, [P, P], bf16)
        colio = load("colio_t", colio_d, [P, P], bf16)
        drA = load("drA_t", drA_d, [P, max(RA, 1)], bf16)
        drB = load("drB_t", drB_d, [P, max(RB, 1)], bf16)
        aff = []
        if not trivial_affine:
            for l in range(2):
                aff.append({k: load(f"{k}{l}_t", d, [P, F]) for k, d in aff_d[l].items()})
        idxA = pers.tile([P, max(RA * 8, 1)], mybir.dt.int16, name="idxA_t")
        nc.sync.dma_start(out=idxA[:], in_=idxA_d[:])
        idxB = pers.tile([P, max(RB * 8, 1)], mybir.dt.int16, name="idxB_t")
        nc.sync.dma_start(out=idxB[:], in_=idxB_d[:])

        hs_self = pers.tile([P, NB, F], bf16, name="hs_self")
        out_prev = pers.tile([P, NB, F], f32, name="out_prev")

        eps_t = pers.tile([P, 1], f32, name="eps_t")
        nc.vector.memset(eps_t[:], LN_EPS)

        # dinv = 1/sqrt(counts + 1)
        dsq = pers.tile([P, NB], f32, name="dsq")
        nc.scalar.activation(out=dsq[:], in_=cnts[:],
                             func=mybir.ActivationFunctionType.Sqrt, bias=1.0)
        dinv = pers.tile([P, NB], f32, name="dinv")
        nc.vector.reciprocal(out=dinv[:], in_=dsq[:])

        # per-layer -sum(b)/128 (for LN mean with bias folded in)
        nsb = []
        if not trivial_affine:
            for l in range(2):
                s = pers.tile([P, 1], f32, name=f"nsb{l}")
                nc.vector.tensor_reduce(out=s[:], in_=aff[l]["bB"][:],
                                        axis=mybir.AxisListType.X,
                                        op=mybir.AluOpType.add, negate=True)
                nc.scalar.mul(out=s[:], in_=s[:], mul=1.0 / F)
                nsb.append(s)


        def phase_a_block(l, b):
                sl = slice(b * P, (b + 1) * P)
                if l == 0:
                    lhsT = xT[:, sl]
                else:
                    pt = tr_ps.tile([P, P], f32, tag="pt", name=f"pt_{l}_{b}")
                    nc.tensor.transpose(out=pt[:], in_=out_prev[:, b, :], identity=ident[:])
                    lt = tpool.tile([P, P], bf16, tag="lt", name=f"lt_{l}_{b}")
                    nc.vector.tensor_copy(out=lt[:], in_=pt[:])
                    lhsT = lt[:]
                ph = ph_ps.tile([P, F], f32, tag="ph", name=f"ph_{l}_{b}")
                nc.tensor.matmul(out=ph[:], lhsT=lhsT, rhs=w_t[l][:], start=True, stop=True)
                nc.scalar.mul(out=hs_self[:, b, :], in_=ph[:], mul=dinv[:, b:b + 1])
                if (b % 7 == 6 or b == NB - 1) and not shared_ag:
                    b0 = (b // 7) * 7
                    nc.sync.dma_start(
                        out=hs_loc[l][:].rearrange("(n p) f -> p n f", p=P)[:, b0:b + 1, :],
                        in_=hs_self[:, b0:b + 1, :])

        def phase_a(l):
            for b in range(NB):
                phase_a_block(l, b)

        def epilogue(l, b, psum):
            """out_blk = relu(LN(dinv*psum + b) * g + be)"""
            has_aff = not trivial_affine
            pre = tpool.tile([P, F], f32, tag="pre", name=f"pre_{l}_{b}")
            rowsum = vpool.tile([P, 1], f32, tag="rs", name=f"rs_{l}_{b}")
            nc.scalar.activation(out=pre[:], in_=psum[:],
                                 func=mybir.ActivationFunctionType.Copy,
                                 scale=dinv[:, b:b + 1], accum_out=rowsum[:])
            if has_aff:
                pre2 = tpool.tile([P, F], f32, tag="pre2", name=f"pre2_{l}_{b}")
                nc.vector.tensor_tensor(out=pre2[:], in0=pre[:], in1=aff[l]["bB"][:],
                                        op=mybir.AluOpType.add)
            else:
                pre2 = pre
            neg_mu = vpool.tile([P, 1], f32, tag="nmu", name=f"nmu_{l}_{b}")
            nc.scalar.activation(out=neg_mu[:], in_=rowsum[:],
                                 func=mybir.ActivationFunctionType.Identity,
                                 bias=(nsb[l][:, :1] if has_aff else 0.0),
                                 scale=-1.0 / F)
            sq = tpool.tile([P, F], f32, tag="sq", name=f"sq_{l}_{b}")
            varsum = vpool.tile([P, 1], f32, tag="vs", name=f"vs_{l}_{b}")
            nc.scalar.activation(out=sq[:], in_=pre2[:],
                                 func=mybir.ActivationFunctionType.Square,
                                 bias=neg_mu[:, :1], accum_out=varsum[:])
            vv = vpool.tile([P, 1], f32, tag="vv", name=f"vv_{l}_{b}")
            nc.scalar.activation(out=vv[:], in_=varsum[:],
                                 func=mybir.ActivationFunctionType.Identity,
                                 bias=eps_t[:, :1], scale=1.0 / F)
            rinv = vpool.tile([P, 1], f32, tag="ri", name=f"ri_{l}_{b}")
            nc.vector.reciprocal(out=rinv[:], in_=vv[:])
            rv = vpool.tile([P, 1], f32, tag="rv", name=f"rv_{l}_{b}")
            nc.scalar.activation(out=rv[:], in_=rinv[:],
                                 func=mybir.ActivationFunctionType.Sqrt)
            bias2 = vpool.tile([P, 1], f32, tag="b2", name=f"b2_{l}_{b}")
            nc.vector.tensor_tensor(out=bias2[:], in0=neg_mu[:], in1=rv[:],
                                    op=mybir.AluOpType.mult)
            if has_aff:
                u = tpool.tile([P, F], f32, tag="u", name=f"u_{l}_{b}")
                nc.scalar.activation(out=u[:], in_=pre2[:],
                                     func=mybir.ActivationFunctionType.Identity,
                                     bias=bias2[:, :1], scale=rv[:, :1])
                v = tpool.tile([P, F], f32, tag="v", name=f"v_{l}_{b}")
                nc.vector.tensor_tensor(out=v[:], in0=u[:], in1=aff[l]["gB"][:],
                                        op=mybir.AluOpType.mult)
                w2_ = tpool.tile([P, F], f32, tag="w2", name=f"w2_{l}_{b}")
                nc.vector.tensor_tensor(out=w2_[:], in0=v[:], in1=aff[l]["beB"][:],
                                        op=mybir.AluOpType.add)
                fin, ffunc, fbias, fscale = w2_, mybir.ActivationFunctionType.Relu, 0.0, 1.0
            else:
                # fused: relu((pre2 - mu) * rv) in one ACT op
                fin, ffunc, fbias, fscale = pre2, mybir.ActivationFunctionType.Relu, bias2[:, :1], rv[:, :1]
            if l == 0:
                nc.scalar.activation(out=out_prev[:, b, :], in_=fin[:],
                                     func=ffunc, bias=fbias, scale=fscale)
            else:
                ot = tpool.tile([P, F], f32, tag="ot", name=f"ot_{b}")
                nc.scalar.activation(out=ot[:], in_=fin[:],
                                     func=ffunc, bias=fbias, scale=fscale)
                nc.sync.dma_start(
                    out=out_d[:].rearrange("(n p) f -> p n f", p=P)[:, b, :],
                    in_=ot[:])

        def phase_b(l, post_block=None):
            gA = _Gather(nc, tc, gpool, gsems, sem_counts, idxA,
                         hs_full[l][0:SPLIT, :], RA, f"A{l}", qrr=qrr)
            gB = _Gather(nc, tc, gpool, gsems, sem_counts, idxB,
                         hs_full[l][SPLIT:NPAD, :], RB, f"B{l}", qrr=qrr)
            if no_gather:
                dumt = gpool.tile([P, RUNS_PER_CHUNK, F], mybir.dt.bfloat16,
                                  tag="gchunk_dum", bufs=1, name=f"dum_{l}")
                nc.vector.memset(dumt[:, 0, :], 0.0)
            for b in range(NB):
                nmm = MA[b] + MB[b]
                psum = ag_ps.tile([P, F], f32, tag="agg", name=f"agg_{l}_{b}")
                nc.tensor.matmul(out=psum[:], lhsT=identb[:], rhs=hs_self[:, b, :],
                                 start=True, stop=(nmm == 0))
                k = 0
                for stream, g, pos, dr in ((0, gA, posA, drA), (1, gB, posB, drB)):
                    for r in range(pos[b], pos[b + 1]):
                        if no_gather:
                            rhs, ci = dumt[:, 0, :], None
                        else:
                            rhs, ci = g.rhs(r)
                        S = spool.tile([P, P], bf16, tag="S", name=f"S_{l}_{b}_{k}")
                        nc.vector.tensor_tensor(
                            out=S[:], in0=dr[:, r:r + 1].to_broadcast([P, P]),
                            in1=colio[:], op=mybir.AluOpType.is_equal)
                        k += 1
                        mm = nc.tensor.matmul(out=psum[:], lhsT=S[:], rhs=rhs,
                                              start=False, stop=(k == nmm))
                        if ci is not None:
                            g.dep(mm, ci)
                epilogue(l, b, psum)
                if post_block is not None:
                    post_block(b)

        for _rep in range(reps):
          hs_loc = ([dram.tile([PPAD, F], bf16, name=f"hs{l}_loc_{_rep}") for l in range(2)]
                    if not shared_ag else [None, None])
          hs_full = [dram.tile([NPAD, F], bf16, name=f"hs{l}_full_{_rep}",
                               addr_space="Shared") for l in range(2)]
          def do_ag(l):
            if shared_ag:
                # write this core's full hs slice into shared hs_full (one
                # inst; Shared tensors allow a single writer), then barrier:
                # Pool waits for local write completion, then a tiny AllGather
                # synchronizes all cores (each core enters only after its
                # writes landed; completion implies everyone's landed).
                ap = hs_full[l][:].rearrange(
                    "(n p) f -> p n f", p=P)[:, 0:NB, :].copy()
                ap.offset = spid[0] * (PPAD * F)
                wr = nc.sync.dma_start(out=ap, in_=hs_self[:, :, :])
                wr.then_inc(wsems[l], 16)
                wsem_tgt[l] += 16
                with tc.tile_critical(no_gpsimd_drain=True):
                    nc.gpsimd.wait_ge(wsems[l], wsem_tgt[l])
                tin = dram.tile([P, 8], f32, name=f"tin_{l}_{_rep}")
                tout = dram.tile([P * NCORES, 8], f32,
                                 name=f"tout_{l}_{_rep}", addr_space="Shared")
                cc = nc.gpsimd.collective_compute(
                    "AllGather", mybir.AluOpType.bypass,
                    ins=[tin[:]], outs=[tout[:]],
                    replica_groups=[list(range(NCORES))])
                cc.then_inc(agsems[l], 1)
                agsem_tgt[l] += 1
                with tc.tile_critical(no_gpsimd_drain=True):
                    nc.gpsimd.wait_ge(agsems[l], agsem_tgt[l])
            elif for_sim or ag_mode == "local":
                nc.sync.dma_start(out=hs_full[l][0:PPAD, :], in_=hs_loc[l][:])
            elif ag_mode == "tinycoll":
                nc.sync.dma_start(out=hs_full[l][0:PPAD, :], in_=hs_loc[l][:])
                tin = dram.tile([P, 8], mybir.dt.float32, name=f"tin_{l}_{_rep}")
                tout = dram.tile([P * NCORES, 8], mybir.dt.float32,
                                 name=f"tout_{l}_{_rep}", addr_space="Shared")
                nc.gpsimd.collective_compute(
                    "AllGather", mybir.AluOpType.bypass,
                    ins=[tin[:]], outs=[tout[:]],
                    replica_groups=[list(range(NCORES))])
            else:
                nc.gpsimd.collective_compute(
                    "AllGather", mybir.AluOpType.bypass,
                    ins=[hs_loc[l][:]], outs=[hs_full[l][:]],
                    replica_groups=[list(range(NCORES))])
          phase_a(0)
          do_ag(0)
          for _pr in range(probe_reloads):
              nc.gpsimd.load_library(_mlp_lib)
          phase_b(0, post_block=lambda b: phase_a_block(1, b))
          do_ag(1)
          phase_b(1)

    nc.compile()
    return nc


# ---------------------------------------------------------------- entry point
AG_MODE = "coll"      # hs exchange: "coll" = NRT AllGather, "shared" = direct
                      # shared-DRAM writes + tiny-collective barrier
_NC_CACHE = {}


def kernel(x, edge_index, W1, b1, g1, be1, W2, b2, g2, be2):
    x = np.asarray(x)
    in_maps, MA, MB, RA, RB, slot_of = _prep(x, edge_index)

    trivial = all(
        (np.all(np.asarray(b) == 0.0) and np.all(np.asarray(g) == 1.0)
         and np.all(np.asarray(be) == 0.0))
        for b, g, be in ((b1, g1, be1), (b2, g2, be2)))

    key = (tuple(MA), tuple(MB), RA, RB, trivial, AG_MODE)
    nc = _NC_CACHE.get(key)
    if nc is None:
        nc = _build(MA, MB, RA, RB, trivial, ag_mode=AG_MODE)
        _NC_CACHE[key] = nc

    ident = np.eye(P, dtype=np.float32)
    colio = np.tile(np.arange(P, dtype=np.float32)[None, :], (P, 1))
    shared = {
        "W1": np.asarray(W1, dtype=np.float32).astype(ml_dtypes.bfloat16),
        "W2": np.asarray(W2, dtype=np.float32).astype(ml_dtypes.bfloat16),
        "ident": ident,
        "identb": ident.astype(ml_dtypes.bfloat16),
        "colio": colio.astype(ml_dtypes.bfloat16),
    }
    if not trivial:
        for l, (b, g, be) in enumerate(((b1, g1, be1), (b2, g2, be2))):
            shared[f"bB{l+1}"] = np.tile(np.asarray(b, np.float32)[None, :], (P, 1))
            shared[f"gB{l+1}"] = np.tile(np.asarray(g, np.float32)[None, :], (P, 1))
            shared[f"beB{l+1}"] = np.tile(np.asarray(be, np.float32)[None, :], (P, 1))
    for m in in_maps:
        m.update(shared)

    res = run_bass_kernel_spmd(nc, in_maps, core_ids=list(range(NCORES)))
    padded = np.concatenate([res.results[c]["out"] for c in range(NCORES)], axis=0)
    return padded[slot_of].astype(np.float32)



# revision 35
# speedup vs baseline: 2.7915x; 2.7915x over previous
"""2-layer GCN (EvolveGCN eval forward) on 8 Trainium2 NeuronCores.

Strategy (graph/data parallel, per sharding hint):
  - Nodes are assigned to 8 cores x 49 blocks x 128 slots by a host-side
    balanced bin-packing (equalizes per-(core,block) edge load for both
    src-index windows, so the SPMD-uniform per-block run counts carry
    minimal padding). Each core owns the edges whose *dst* lands in its
    slots and produces those output rows.
  - GCN layer is factored as  out = dinv * (segsum(hs[src]) + hs) + b  with
    hs = dinv * (X @ W), so no per-edge FP math is needed beyond the
    segment-sum itself.
  - Per core/layer: dense X@W on TensorE; hs rows are AllGathered across
    cores (bf16 rows in shared DRAM); per-dst-block aggregation runs as
    [128-edge] x [one-hot S] matmuls accumulating in PSUM; LayerNorm+ReLU
    epilogue on VectorE/ScalarE.
  - The per-edge gather of hs[src] uses the GPSIMD dma_gather custom DMA
    (int16 indices; the padded node space 50176 is split in two 25088-row
    windows so indices fit in int16). Gather chunks are striped across 4
    SWDGE queues (one DMA ring each) with deep prefetch: a single ring is
    latency/bandwidth-bound at ~10 ns per 256 B row, so 4 queues give ~4x.
    Edges are sorted by src slot within each run for HBM page locality,
    and padded lanes point at spread-out rows to avoid bank hammering.

Host-side work is integer graph preprocessing only (edge bucketing/sorting,
balanced slot assignment, index relabeling, layout transposes); all
floating-point model math runs on device.
"""
import sys

sys.path.insert(0, "/opt/trn_rl_repo")

import numpy as np
import ml_dtypes

import concourse.bacc as bacc
import concourse.bass as bass
import concourse.mybir as mybir
import concourse.tile as tile
from concourse.tile import add_dep_helper
from concourse.bass_utils import run_bass_kernel_spmd
from concourse.library_config import mlp as _mlp_lib
from concourse._compat import cdiv

P = 128
NCORES = 8
N_NODES = 50000
PART = N_NODES // NCORES          # 6250 real nodes per core
NB = cdiv(PART, P)                # 49 dst blocks per core
PPAD = NB * P                     # 6272 padded nodes per core
NPAD = NCORES * PPAD              # 50176 padded global nodes
SPLIT = NPAD // 2                 # 25088 (< 32768 so int16 indices work)
F = 128                           # feature dim
LN_EPS = 1e-5
RUNS_PER_CHUNK = 32               # 32 runs * 128 edges = 4096 gathers/call
GPOOL_BUFS = 1                    # per-tag bufs set at tile() time
SPOOL_BUFS = 16
N_GSEMS = 8
N_QUEUES = 4                      # SWDGE queues; chunks striped across them


# ---------------------------------------------------------------- host prep
def _wrap16(arr):
    """[L] int -> [128, L/16] int16, index j of the stream at [j%16, j//16],
    replicated to all 8 GPSIMD core partition groups."""
    L = arr.shape[0]
    assert L % 16 == 0
    a2 = arr.reshape(L // 16, 16).T          # [16, S]
    return np.ascontiguousarray(np.tile(a2, (8, 1)).astype(np.int16))


def _balance_slots(a_n, b_n):
    """Assign each node a padded slot, balancing per-(core,block) A/B edge
    loads. Nodes < N_NODES//2 go to slots < SPLIT (cores 0-3) so the A/B
    src-window classification stays consistent. Returns slot_of [N_NODES]."""
    half = N_NODES // 2
    NBINS = NCORES * NB
    A = np.zeros(NBINS, dtype=np.int64)
    B = np.zeros(NBINS, dtype=np.int64)
    CNT = np.zeros(NBINS, dtype=np.int64)
    slot_of = np.empty(N_NODES, dtype=np.int64)
    half_bins = NBINS // 2
    deg = a_n + b_n
    for lo, hi, bins in ((0, half, slice(0, half_bins)),
                         (half, N_NODES, slice(half_bins, NBINS))):
        nodes = np.arange(lo, hi)
        order = nodes[np.argsort(-deg[lo:hi], kind="stable")]
        Av, Bv, Cv = A[bins], B[bins], CNT[bins]
        base = (bins.start if bins.start else 0)
        cap = 6.0 * P  # 768: keep both streams at <=6 runs per block
        for n in order:
            An, Bn = Av + a_n[n], Bv + b_n[n]
            score = (np.maximum(An - cap, 0.0) ** 2
                     + np.maximum(Bn - cap, 0.0) ** 2
                     + 1e-6 * (An * An + Bn * Bn))
            score[Cv >= P] = np.inf
            i = int(np.argmin(score))
            slot_of[n] = (base + i) * P + Cv[i]
            Av[i] += a_n[n]
            Bv[i] += b_n[n]
            Cv[i] += 1
    return slot_of


def _prep(x, edge_index):
    """Integer-only graph preprocessing + layout prep. Returns
    (per-core input dict list, MA, MB, RA, RB, slot_of) where MA/MB are
    per-block run counts (uniform across cores -> baked into the program)
    and slot_of maps node id -> padded slot (for x/out permutation)."""
    src = np.asarray(edge_index[0], dtype=np.int64)
    dst = np.asarray(edge_index[1], dtype=np.int64)

    half = N_NODES // 2
    a_cnt = np.bincount(dst[src < half], minlength=N_NODES)
    b_cnt = np.bincount(dst[src >= half], minlength=N_NODES)
    slot_of = _balance_slots(a_cnt, b_cnt)

    counts = (a_cnt + b_cnt).astype(np.float32)

    sd = slot_of[dst]
    core_of = sd // PPAD
    blk = (sd % PPAD) >> 7                   # 0..48
    dr = sd & (P - 1)                        # position within block
    ps = slot_of[src]                        # padded global src slot
    is_b = (ps >= SPLIT).astype(np.int64)

    order = np.lexsort((ps, is_b, blk, core_of))
    ps_s, dr_s, blk_s, core_s, isb_s = (
        ps[order], dr[order], blk[order], core_of[order], is_b[order])

    # group sizes per (core, blk, range)
    key = (core_s * NB + blk_s) * 2 + isb_s
    sizes = np.bincount(key, minlength=NCORES * NB * 2).reshape(NCORES, NB, 2)
    MA = np.max(np.ceil(sizes[:, :, 0] / P).astype(np.int64), axis=0)  # [NB]
    MB = np.max(np.ceil(sizes[:, :, 1] / P).astype(np.int64), axis=0)  # [NB]
    posA = np.concatenate([[0], np.cumsum(MA)])   # run offset of block b in A stream
    posB = np.concatenate([[0], np.cumsum(MB)])
    RA, RB = int(posA[-1]), int(posB[-1])

    # per-core edge segments (cumulative offsets into the sorted arrays)
    seg_end = np.cumsum(sizes.reshape(-1))
    seg_start = seg_end - sizes.reshape(-1)
    seg_start = seg_start.reshape(NCORES, NB, 2)
    seg_len = sizes

    # spread padding indices over distinct rows (avoid HBM bank hammering)
    padA = (np.arange(RA * P, dtype=np.int64) * 37) % SPLIT
    padB = (np.arange(RB * P, dtype=np.int64) * 37) % (NPAD - SPLIT)

    in_maps = []
    for c in range(NCORES):
        idxA = padA.copy()
        drA = np.full(RA * P, -1.0, dtype=np.float32)
        idxB = padB.copy()
        drB = np.full(RB * P, -1.0, dtype=np.float32)
        for b in range(NB):
            s0, n0 = seg_start[c, b, 0], seg_len[c, b, 0]
            o = posA[b] * P
            idxA[o:o + n0] = ps_s[s0:s0 + n0]
            drA[o:o + n0] = dr_s[s0:s0 + n0].astype(np.float32)
            s1, n1 = seg_start[c, b, 1], seg_len[c, b, 1]
            o = posB[b] * P
            idxB[o:o + n1] = ps_s[s1:s1 + n1] - SPLIT
            drB[o:o + n1] = dr_s[s1:s1 + n1].astype(np.float32)

        cnt = np.zeros(NPAD, dtype=np.float32)
        cnt[slot_of] = counts
        cnt = cnt[c * PPAD:(c + 1) * PPAD]
        xs = np.zeros((NPAD, F), dtype=np.float32)
        xs[slot_of] = np.asarray(x, dtype=np.float32)
        xT = np.ascontiguousarray(
            xs[c * PPAD:(c + 1) * PPAD].T).astype(ml_dtypes.bfloat16)

        m = {
            "xT": np.ascontiguousarray(xT),
            "cnts": np.ascontiguousarray(cnt.reshape(NB, P).T),       # [128, NB]
            "idxA": _wrap16(idxA) if RA else np.zeros((128, 1), np.int16),
            "idxB": _wrap16(idxB) if RB else np.zeros((128, 1), np.int16),
            "drA": (np.ascontiguousarray(drA.reshape(RA, P).T).astype(ml_dtypes.bfloat16)
                    if RA else np.zeros((128, 1), ml_dtypes.bfloat16)),
            "drB": (np.ascontiguousarray(drB.reshape(RB, P).T).astype(ml_dtypes.bfloat16)
                    if RB else np.zeros((128, 1), ml_dtypes.bfloat16)),
        }
        in_maps.append(m)
    return in_maps, MA.tolist(), MB.tolist(), RA, RB, slot_of


# ---------------------------------------------------------------- device code
def _inst(x):
    # unwrap BassInstruction -> mybir.Instruction; raw Instructions pass through
    # (note: Instruction.ins is its *operand list*, so type-check, don't duck-type)
    import bass_rust as _br
    if isinstance(x, _br.Instruction):
        return x
    return x.ins


class _Gather:
    """Streams one range's gather chunks through a tile pool with software
    prefetch: the dma_gather issue and the data-arrival wait live in separate
    critical sections so descriptor generation and DMA of later chunks overlap
    the wait for the current one. Consumers must register via dep()."""

    PREFETCH = 5

    def __init__(self, nc, tc, pool, sems, sem_counts, idx_t, src_ap, n_runs, name,
                 qrr=None):
        self.nc, self.tc, self.pool = nc, tc, pool
        self.sems, self.sem_counts = sems, sem_counts
        self.idx_t, self.src_ap, self.n_runs = idx_t, src_ap, n_runs
        self.n_chunks = cdiv(n_runs, RUNS_PER_CHUNK)
        self.name = name
        self.qrr = qrr if qrr is not None else [0]   # shared round-robin counter
        self.issued = -1
        self.sem_i = 0
        self.tiles = {}
        self.gi = {}        # chunk -> gather inst
        self.wi = {}        # chunk -> wait inst
        self.semtgt = {}

    def _emit_issue(self, ci):
        nruns = min(RUNS_PER_CHUNK, self.n_runs - ci * RUNS_PER_CHUNK)
        n = nruns * P
        off = ci * RUNS_PER_CHUNK * P
        g = self.pool.tile([P, RUNS_PER_CHUNK, F], mybir.dt.bfloat16,
                           tag=f"gchunk_{self.name[:1]}", bufs=self.PREFETCH + 2,
                           name=f"g_{self.name}_{ci}")
        k = self.sem_i % len(self.sems)
        self.sem_i += 1
        sem = self.sems[k]
        self.sem_counts[k] += 16
        self.semtgt[ci] = (sem, self.sem_counts[k])
        qn = self.qrr[0] % N_QUEUES
        self.qrr[0] += 1
        gi = self.nc.gpsimd.dma_gather(
            g[:, :nruns, :], self.src_ap,
            self.idx_t[:, off // 16:(off + n) // 16],
            n, n, F, single_packet=False, queue_num=qn,
        ).then_inc(sem, 16)
        self.tiles[ci] = g
        self.gi[ci] = gi
        self.issued = ci

    def rhs(self, r):
        """Returns (rhs AP for run r, chunk index). Caller must dep() every
        consumer instruction on that chunk. One critical section per chunk
        advance: {issue chunk ci+PREFETCH; wait chunk ci} -- sections chain
        in emission order on gpsimd, so issues always precede their waits."""
        ci = r // RUNS_PER_CHUNK
        if ci not in self.wi:
            with self.tc.tile_critical(no_gpsimd_drain=True):
                while self.issued < min(ci + self.PREFETCH, self.n_chunks - 1):
                    self._emit_issue(self.issued + 1)
                sem, tgt = self.semtgt[ci]
                self.nc.gpsimd.wait_ge(sem, tgt)
            self.wi[ci] = self.tc.prev_crit_insts[mybir.EngineType.Pool]
            self.tiles.pop(ci - 1, None)
        return self.tiles[ci][:, r - ci * RUNS_PER_CHUNK, :], ci

    def dep(self, inst, ci):
        add_dep_helper(_inst(inst), _inst(self.wi[ci]), sync=True,
                       reason=f"gather consume {self.name}:{ci}")


def _build(MA, MB, RA, RB, trivial_affine, for_sim=False, reps=1, no_gather=False,
           ag_mode="coll", probe_reloads=0):
    """Build + compile the 8-core SPMD program. trivial_affine: b==0, g==1,
    be==0 for both layers (checked on host) -> skip those tensor ops.
    for_sim: single-core cost-model variant (AllGather replaced by a local
    DRAM copy so TimelineSim can run it)."""
    posA = np.concatenate([[0], np.cumsum(MA)]).astype(int)
    posB = np.concatenate([[0], np.cumsum(MB)]).astype(int)

    nc = bacc.Bacc("TRN2", target_bir_lowering=False, debug=False,
                   num_devices=1 if for_sim else NCORES,
                   num_swdge_queues=N_QUEUES)
    qrr = [0]
    f32 = mybir.dt.float32
    bf16 = mybir.dt.bfloat16

    # ---- I/O
    xT_d = nc.dram_tensor("xT", [F, PPAD], mybir.dt.bfloat16, kind="ExternalInput")
    cnts_d = nc.dram_tensor("cnts", [P, NB], f32, kind="ExternalInput")
    idxA_d = nc.dram_tensor("idxA", [P, max(RA * 8, 1)], mybir.dt.int16, kind="ExternalInput")
    idxB_d = nc.dram_tensor("idxB", [P, max(RB * 8, 1)], mybir.dt.int16, kind="ExternalInput")
    drA_d = nc.dram_tensor("drA", [P, max(RA, 1)], mybir.dt.bfloat16, kind="ExternalInput")
    drB_d = nc.dram_tensor("drB", [P, max(RB, 1)], mybir.dt.bfloat16, kind="ExternalInput")
    w_d = [nc.dram_tensor(f"W{l+1}", [F, F], mybir.dt.bfloat16, kind="ExternalInput") for l in range(2)]
    aff_d = []
    if not trivial_affine:
        for l in range(2):
            aff_d.append({k: nc.dram_tensor(f"{k}{l+1}", [P, F], f32, kind="ExternalInput")
                          for k in ("bB", "gB", "beB")})
    ident_d = nc.dram_tensor("ident", [P, P], f32, kind="ExternalInput")
    identb_d = nc.dram_tensor("identb", [P, P], mybir.dt.bfloat16, kind="ExternalInput")
    colio_d = nc.dram_tensor("colio", [P, P], mybir.dt.bfloat16, kind="ExternalInput")
    out_d = nc.dram_tensor("out", [PPAD, F], f32, kind="ExternalOutput")

    shared_ag = (ag_mode == "shared") and not for_sim
    shared2 = (ag_mode == "shared2") and not for_sim

    import contextlib
    with tile.TileContext(nc) as tc, contextlib.ExitStack() as st:
        gsems = [st.enter_context(nc.semaphore(f"gsem{i}")) for i in range(N_GSEMS)]
        sem_counts = [0] * N_GSEMS
        wsems = agsems = None
        wsem_tgt = [0, 0]
        agsem_tgt = [0, 0]
        spid = [None]
        if shared_ag or shared2:
            wsems = [st.enter_context(nc.semaphore(f"wsem{l}")) for l in range(2)]
            agsems = [st.enter_context(nc.semaphore(f"agsem{l}")) for l in range(2)]
        hs_sh = None
        if shared2:
            # raw Shared DRAM tensors (not pool tiles): written by 8
            # statically-addressed predicated DMAs (cond=pid==c; skipped DMAs
            # still bump the completion sem), read by the gathers. Reuse
            # across reps is safe: each layer's barrier implies every core
            # finished the prior rep's reads of that buffer.
            hs_sh = [nc.dram_tensor(f"hs_sh{l}", [NPAD, F], mybir.dt.bfloat16,
                                    kind="Internal", addr_space="Shared")
                     for l in range(2)]
        pers = st.enter_context(tc.tile_pool(name="pers", bufs=1))
        gpool = st.enter_context(tc.tile_pool(name="gpool", bufs=GPOOL_BUFS))
        spool = st.enter_context(tc.tile_pool(name="spool", bufs=SPOOL_BUFS))
        tpool = st.enter_context(tc.tile_pool(name="tpool", bufs=4))
        vpool = st.enter_context(tc.tile_pool(name="vpool", bufs=8))
        ph_ps = st.enter_context(tc.tile_pool(name="ph_ps", bufs=2, space="PSUM"))
        ag_ps = st.enter_context(tc.tile_pool(name="ag_ps", bufs=5, space="PSUM"))
        tr_ps = st.enter_context(tc.tile_pool(name="tr_ps", bufs=1, space="PSUM"))
        dram = st.enter_context(tc.tile_pool(name="dram", bufs=1, space="DRAM"))

        nc.gpsimd.load_library(_mlp_lib)
        if shared_ag or shared2:
            spid[0] = nc.sync.partition_id()

        # ---- persistent loads
        def load(name, dten, shape, dt=None):
            t = pers.tile(shape, dt or f32, name=name)
            nc.sync.dma_start(out=t[:], in_=dten[:])
            return t

        xT = load("xT_t", xT_d, [F, PPAD], bf16)
        cnts = load("cnts_t", cnts_d, [P, NB])
        w_t = [load(f"w{l}_t", w_d[l], [F, F], bf16) for l in range(2)]
        ident = load("ident_t", ident_d, [P, P])
        identb = load("identb_t", identb_d, [P, P], bf16)
        colio = load("colio_t", colio_d, [P, P], bf16)
        drA = load("drA_t", drA_d, [P, max(RA, 1)], bf16)
        drB = load("drB_t", drB_d, [P, max(RB, 1)], bf16)
        aff = []
        if not trivial_affine:
            for l in range(2):
                aff.append({k: load(f"{k}{l}_t", d, [P, F]) for k, d in aff_d[l].items()})
        idxA = pers.tile([P, max(RA * 8, 1)], mybir.dt.int16, name="idxA_t")
        nc.sync.dma_start(out=idxA[:], in_=idxA_d[:])
        idxB = pers.tile([P, max(RB * 8, 1)], mybir.dt.int16, name="idxB_t")
        nc.sync.dma_start(out=idxB[:], in_=idxB_d[:])

        hs_self = pers.tile([P, NB, F], bf16, name="hs_self")
        out_prev = pers.tile([P, NB, F], f32, name="out_prev")

        eps_t = pers.tile([P, 1], f32, name="eps_t")
        nc.vector.memset(eps_t[:], LN_EPS)

        # dinv = 1/sqrt(counts + 1)
        dsq = pers.tile([P, NB], f32, name="dsq")
        nc.scalar.activation(out=dsq[:], in_=cnts[:],
                             func=mybir.ActivationFunctionType.Sqrt, bias=1.0)
        dinv = pers.tile([P, NB], f32, name="dinv")
        nc.vector.reciprocal(out=dinv[:], in_=dsq[:])

        # per-layer -sum(b)/128 (for LN mean with bias folded in)
        nsb = []
        if not trivial_affine:
            for l in range(2):
                s = pers.tile([P, 1], f32, name=f"nsb{l}")
                nc.vector.tensor_reduce(out=s[:], in_=aff[l]["bB"][:],
                                        axis=mybir.AxisListType.X,
                                        op=mybir.AluOpType.add, negate=True)
                nc.scalar.mul(out=s[:], in_=s[:], mul=1.0 / F)
                nsb.append(s)


        def phase_a_block(l, b):
                sl = slice(b * P, (b + 1) * P)
                if l == 0:
                    lhsT = xT[:, sl]
                else:
                    pt = tr_ps.tile([P, P], f32, tag="pt", name=f"pt_{l}_{b}")
                    nc.tensor.transpose(out=pt[:], in_=out_prev[:, b, :], identity=ident[:])
                    lt = tpool.tile([P, P], bf16, tag="lt", name=f"lt_{l}_{b}")
                    nc.vector.tensor_copy(out=lt[:], in_=pt[:])
                    lhsT = lt[:]
                ph = ph_ps.tile([P, F], f32, tag="ph", name=f"ph_{l}_{b}")
                nc.tensor.matmul(out=ph[:], lhsT=lhsT, rhs=w_t[l][:], start=True, stop=True)
                nc.scalar.mul(out=hs_self[:, b, :], in_=ph[:], mul=dinv[:, b:b + 1])
                if (b % 7 == 6 or b == NB - 1) and not (shared_ag or shared2):
                    b0 = (b // 7) * 7
                    nc.sync.dma_start(
                        out=hs_loc[l][:].rearrange("(n p) f -> p n f", p=P)[:, b0:b + 1, :],
                        in_=hs_self[:, b0:b + 1, :])

        def phase_a(l):
            for b in range(NB):
                phase_a_block(l, b)

        def epilogue(l, b, psum):
            """out_blk = relu(LN(dinv*psum + b) * g + be)"""
            has_aff = not trivial_affine
            pre = tpool.tile([P, F], f32, tag="pre", name=f"pre_{l}_{b}")
            rowsum = vpool.tile([P, 1], f32, tag="rs", name=f"rs_{l}_{b}")
            nc.scalar.activation(out=pre[:], in_=psum[:],
                                 func=mybir.ActivationFunctionType.Copy,
                                 scale=dinv[:, b:b + 1], accum_out=rowsum[:])
            if has_aff:
                pre2 = tpool.tile([P, F], f32, tag="pre2", name=f"pre2_{l}_{b}")
                nc.vector.tensor_tensor(out=pre2[:], in0=pre[:], in1=aff[l]["bB"][:],
                                        op=mybir.AluOpType.add)
            else:
                pre2 = pre
            neg_mu = vpool.tile([P, 1], f32, tag="nmu", name=f"nmu_{l}_{b}")
            nc.scalar.activation(out=neg_mu[:], in_=rowsum[:],
                                 func=mybir.ActivationFunctionType.Identity,
                                 bias=(nsb[l][:, :1] if has_aff else 0.0),
                                 scale=-1.0 / F)
            sq = tpool.tile([P, F], f32, tag="sq", name=f"sq_{l}_{b}")
            varsum = vpool.tile([P, 1], f32, tag="vs", name=f"vs_{l}_{b}")
            nc.scalar.activation(out=sq[:], in_=pre2[:],
                                 func=mybir.ActivationFunctionType.Square,
                                 bias=neg_mu[:, :1], accum_out=varsum[:])
            vv = vpool.tile([P, 1], f32, tag="vv", name=f"vv_{l}_{b}")
            nc.scalar.activation(out=vv[:], in_=varsum[:],
                                 func=mybir.ActivationFunctionType.Identity,
                                 bias=eps_t[:, :1], scale=1.0 / F)
            rinv = vpool.tile([P, 1], f32, tag="ri", name=f"ri_{l}_{b}")
            nc.vector.reciprocal(out=rinv[:], in_=vv[:])
            rv = vpool.tile([P, 1], f32, tag="rv", name=f"rv_{l}_{b}")
            nc.scalar.activation(out=rv[:], in_=rinv[:],
                                 func=mybir.ActivationFunctionType.Sqrt)
            bias2 = vpool.tile([P, 1], f32, tag="b2", name=f"b2_{l}_{b}")
            nc.vector.tensor_tensor(out=bias2[:], in0=neg_mu[:], in1=rv[:],
                                    op=mybir.AluOpType.mult)
            if has_aff:
                u = tpool.tile([P, F], f32, tag="u", name=f"u_{l}_{b}")
                nc.scalar.activation(out=u[:], in_=pre2[:],
                                     func=mybir.ActivationFunctionType.Identity,
                                     bias=bias2[:, :1], scale=rv[:, :1])
                v = tpool.tile([P, F], f32, tag="v", name=f"v_{l}_{b}")
                nc.vector.tensor_tensor(out=v[:], in0=u[:], in1=aff[l]["gB"][:],
                                        op=mybir.AluOpType.mult)
                w2_ = tpool.tile([P, F], f32, tag="w2", name=f"w2_{l}_{b}")
                nc.vector.tensor_tensor(out=w2_[:], in0=v[:], in1=aff[l]["beB"][:],
                                        op=mybir.AluOpType.add)
                fin, ffunc, fbias, fscale = w2_, mybir.ActivationFunctionType.Relu, 0.0, 1.0
            else:
                # fused: relu((pre2 - mu) * rv) in one ACT op
                fin, ffunc, fbias, fscale = pre2, mybir.ActivationFunctionType.Relu, bias2[:, :1], rv[:, :1]
            if l == 0:
                nc.scalar.activation(out=out_prev[:, b, :], in_=fin[:],
                                     func=ffunc, bias=fbias, scale=fscale)
            else:
                ot = tpool.tile([P, F], f32, tag="ot", name=f"ot_{b}")
                nc.scalar.activation(out=ot[:], in_=fin[:],
                                     func=ffunc, bias=fbias, scale=fscale)
                nc.sync.dma_start(
                    out=out_d[:].rearrange("(n p) f -> p n f", p=P)[:, b, :],
                    in_=ot[:])

        def phase_b(l, post_block=None):
            gA = _Gather(nc, tc, gpool, gsems, sem_counts, idxA,
                         hs_full[l][0:SPLIT, :], RA, f"A{l}", qrr=qrr)
            gB = _Gather(nc, tc, gpool, gsems, sem_counts, idxB,
                         hs_full[l][SPLIT:NPAD, :], RB, f"B{l}", qrr=qrr)
            if no_gather:
                dumt = gpool.tile([P, RUNS_PER_CHUNK, F], mybir.dt.bfloat16,
                                  tag="gchunk_dum", bufs=1, name=f"dum_{l}")
                nc.vector.memset(dumt[:, 0, :], 0.0)
            for b in range(NB):
                nmm = MA[b] + MB[b]
                psum = ag_ps.tile([P, F], f32, tag="agg", name=f"agg_{l}_{b}")
                nc.tensor.matmul(out=psum[:], lhsT=identb[:], rhs=hs_self[:, b, :],
                                 start=True, stop=(nmm == 0))
                k = 0
                for stream, g, pos, dr in ((0, gA, posA, drA), (1, gB, posB, drB)):
                    for r in range(pos[b], pos[b + 1]):
                        if no_gather:
                            rhs, ci = dumt[:, 0, :], None
                        else:
                            rhs, ci = g.rhs(r)
                        S = spool.tile([P, P], bf16, tag="S", name=f"S_{l}_{b}_{k}")
                        nc.vector.tensor_tensor(
                            out=S[:], in0=dr[:, r:r + 1].to_broadcast([P, P]),
                            in1=colio[:], op=mybir.AluOpType.is_equal)
                        k += 1
                        mm = nc.tensor.matmul(out=psum[:], lhsT=S[:], rhs=rhs,
                                              start=False, stop=(k == nmm))
                        if ci is not None:
                            g.dep(mm, ci)
                epilogue(l, b, psum)
                if post_block is not None:
                    post_block(b)

        for _rep in range(reps):
          hs_loc = ([dram.tile([PPAD, F], bf16, name=f"hs{l}_loc_{_rep}") for l in range(2)]
                    if not (shared_ag or shared2) else [None, None])
          if shared2:
              hs_full = hs_sh
          else:
              hs_full = [dram.tile([NPAD, F], bf16, name=f"hs{l}_full_{_rep}",
                                   addr_space="Shared") for l in range(2)]
          def do_ag(l):
            if shared2:
                # 8 statically-addressed predicated writes: only the one with
                # cond pid==c executes; the rest are skipped but still bump
                # the completion sem, so every core waits for 8*16.
                for c in range(NCORES):
                    ap = hs_sh[l][:].rearrange(
                        "(n p) f -> p n f", p=P)[:, c * NB:(c + 1) * NB, :]
                    wr = nc.sync.dma_start(out=ap, in_=hs_self[:, :, :],
                                           cond=spid[0] == c, cond_hint=False)
                    wr.then_inc(wsems[l], 16)
                    wsem_tgt[l] += 16
                with tc.tile_critical(no_gpsimd_drain=True):
                    nc.gpsimd.wait_ge(wsems[l], wsem_tgt[l])
                tin = dram.tile([P, 8], f32, name=f"tin_{l}_{_rep}")
                tout = dram.tile([P * NCORES, 8], f32,
                                 name=f"tout_{l}_{_rep}", addr_space="Shared")
                cc = nc.gpsimd.collective_compute(
                    "AllGather", mybir.AluOpType.bypass,
                    ins=[tin[:]], outs=[tout[:]],
                    replica_groups=[list(range(NCORES))])
                # anchor Pool program order after the collective completes
                # (codegen rejects then_inc on collectives)
                with tc.tile_critical(no_gpsimd_drain=True):
                    nc.gpsimd.wait_ge(agsems[l], 0)
                bar = tc.prev_crit_insts[mybir.EngineType.Pool]
                add_dep_helper(_inst(bar), _inst(cc), sync=True,
                               reason="tiny-AG barrier")
            elif shared_ag:
                # write this core's full hs slice into shared hs_full (one
                # inst; Shared tensors allow a single writer), then barrier:
                # Pool waits for local write completion, then a tiny AllGather
                # synchronizes all cores (each core enters only after its
                # writes landed; completion implies everyone's landed).
                ap = hs_full[l][:].rearrange(
                    "(n p) f -> p n f", p=P)[:, 0:NB, :].copy()
                ap.offset = spid[0] * (PPAD * F)
                wr = nc.sync.dma_start(out=ap, in_=hs_self[:, :, :])
                wr.then_inc(wsems[l], 16)
                wsem_tgt[l] += 16
                with tc.tile_critical(no_gpsimd_drain=True):
                    nc.gpsimd.wait_ge(wsems[l], wsem_tgt[l])
                tin = dram.tile([P, 8], f32, name=f"tin_{l}_{_rep}")
                tout = dram.tile([P * NCORES, 8], f32,
                                 name=f"tout_{l}_{_rep}", addr_space="Shared")
                cc = nc.gpsimd.collective_compute(
                    "AllGather", mybir.AluOpType.bypass,
                    ins=[tin[:]], outs=[tout[:]],
                    replica_groups=[list(range(NCORES))])
                cc.then_inc(agsems[l], 1)
                agsem_tgt[l] += 1
                with tc.tile_critical(no_gpsimd_drain=True):
                    nc.gpsimd.wait_ge(agsems[l], agsem_tgt[l])
            elif for_sim or ag_mode == "local":
                nc.sync.dma_start(out=hs_full[l][0:PPAD, :], in_=hs_loc[l][:])
            elif ag_mode == "tinycoll":
                nc.sync.dma_start(out=hs_full[l][0:PPAD, :], in_=hs_loc[l][:])
                tin = dram.tile([P, 8], mybir.dt.float32, name=f"tin_{l}_{_rep}")
                tout = dram.tile([P * NCORES, 8], mybir.dt.float32,
                                 name=f"tout_{l}_{_rep}", addr_space="Shared")
                nc.gpsimd.collective_compute(
                    "AllGather", mybir.AluOpType.bypass,
                    ins=[tin[:]], outs=[tout[:]],
                    replica_groups=[list(range(NCORES))])
            else:
                nc.gpsimd.collective_compute(
                    "AllGather", mybir.AluOpType.bypass,
                    ins=[hs_loc[l][:]], outs=[hs_full[l][:]],
                    replica_groups=[list(range(NCORES))])
          phase_a(0)
          do_ag(0)
          for _pr in range(probe_reloads):
              nc.gpsimd.load_library(_mlp_lib)
          phase_b(0, post_block=lambda b: phase_a_block(1, b))
          do_ag(1)
          phase_b(1)

    nc.compile()
    return nc


# ---------------------------------------------------------------- entry point
AG_MODE = "coll"      # hs exchange: "coll" = NRT AllGather, "shared" = direct
                      # shared-DRAM writes + tiny-collective barrier
_NC_CACHE = {}


def kernel(x, edge_index, W1, b1, g1, be1, W2, b2, g2, be2):
    x = np.asarray(x)
    in_maps, MA, MB, RA, RB, slot_of = _prep(x, edge_index)

    trivial = all(
        (np.all(np.asarray(b) == 0.0) and np.all(np.asarray(g) == 1.0)
         and np.all(np.asarray(be) == 0.0))
        for b, g, be in ((b1, g1, be1), (b2, g2, be2)))

    key = (tuple(MA), tuple(MB), RA, RB, trivial, AG_MODE)
    nc = _NC_CACHE.get(key)
    if nc is None:
        nc = _build(MA, MB, RA, RB, trivial, ag_mode=AG_MODE)
        _NC_CACHE[key] = nc

    ident = np.eye(P, dtype=np.float32)
    colio = np.tile(np.arange(P, dtype=np.float32)[None, :], (P, 1))
    shared = {
        "W1": np.asarray(W1, dtype=np.float32).astype(ml_dtypes.bfloat16),
        "W2": np.asarray(W2, dtype=np.float32).astype(ml_dtypes.bfloat16),
        "ident": ident,
        "identb": ident.astype(ml_dtypes.bfloat16),
        "colio": colio.astype(ml_dtypes.bfloat16),
    }
    if not trivial:
        for l, (b, g, be) in enumerate(((b1, g1, be1), (b2, g2, be2))):
            shared[f"bB{l+1}"] = np.tile(np.asarray(b, np.float32)[None, :], (P, 1))
            shared[f"gB{l+1}"] = np.tile(np.asarray(g, np.float32)[None, :], (P, 1))
            shared[f"beB{l+1}"] = np.tile(np.asarray(be, np.float32)[None, :], (P, 1))
    for m in in_maps:
        m.update(shared)

    res = run_bass_kernel_spmd(nc, in_maps, core_ids=list(range(NCORES)))
    padded = np.concatenate([res.results[c]["out"] for c in range(NCORES)], axis=0)
    return padded[slot_of].astype(np.float32)

